# revision 19
# baseline (speedup 1.0000x reference)
"""Trainium2 Bass kernel for multi-head attention (B=2, S=2048, D=1024, H=16).

Sharding: tensor-parallel over heads — each of 8 cores owns 2 heads (both
batches).  Host pre-transposes x to xT [D, B*S] so every device matmul gets
operands in natural layout (no on-device transposes):

  per core (heads 2c, 2c+1):
    qT,kT [128, BS]  = w_{q,k}colsT @ x   (lhsT = w cols  [D,128], rhs = xT)
    v     [BS, 128]  = x @ w_vcols        (lhsT = xT tile [D,128], rhs = w_v)
    scoresT [ki,q]   = k qT               (lhsT = kT 64-row blocks, rhs = qT;
                                           2 heads packed via PE row-tiling)
    exp on ScalarE straight from PSUM, no max subtraction (scores ~ N(0,1))
    aoT_un [128f, q] = v^T exp            (2 heads packed via PE col-tiling)
    colsum [1, q]    = ones^T exp         (concurrent M=1 matmuls)
    aoT = aoT_un * (1/colsum broadcast)   (gpsimd partition_broadcast + DVE)
    yT_partial [D, BS] = w_out_rowsT @ aoT

  host: y = (sum_c yT_partial_c).T.reshape(B,S,D) + b_out
"""

import os
import sys

for _p in ("/opt/trn_rl_repo",):
    if _p not in sys.path and os.path.isdir(_p):
        sys.path.insert(0, _p)

import numpy as np

# Problem shapes (hardcoded per contest rules).
B, S, D, H = 2, 2048, 1024, 16
DH = D // H            # 64
NCORES = 8
HPC = H // NCORES      # 2 heads per core
FP = HPC * DH          # 128 features per core
BS = B * S             # 4096 tokens
KT = D // 128          # 8 contraction k-tiles
QW = 512               # attention q-slice width (one PSUM bank of fp32)


def build_program(b=B, s=S, d=D, h=H, ncores=NCORES, debug_outputs=False):
    """Build + compile the per-core Bass program. Same program for all cores
    (SPMD); per-core differences come in through the weight slices."""
    import concourse.bass as bass
    import concourse.mybir as mybir
    import concourse.tile as tile
    from concourse import bacc

    dh = d // h
    hpc = h // ncores
    fp = hpc * dh                      # per-core features (q/k/v width)
    bs = b * s
    kt = d // 128                      # k tiles over model dim
    qw = min(QW, s)                    # q-slice width
    nqs = s // qw                      # q slices per batch
    ki_n = s // 128                    # key blocks per batch
    vmb = s // 128                     # v m-blocks per batch
    nms = s // 512 if s >= 512 else 1  # qkv m-slices per batch
    msw = min(512, s)                  # m-slice width
    assert fp == 128 and dh == 64

    f32 = mybir.dt.float32
    AF = mybir.ActivationFunctionType

    nc = bacc.Bacc("TRN2", target_bir_lowering=False, debug=False)

    xT_d = nc.dram_tensor("xT", [d, bs], f32, kind="ExternalInput")
    wq_d = nc.dram_tensor("wq", [d, fp], f32, kind="ExternalInput")
    wk_d = nc.dram_tensor("wk", [d, fp], f32, kind="ExternalInput")
    wv_d = nc.dram_tensor("wv", [d, fp], f32, kind="ExternalInput")
    wo_d = nc.dram_tensor("wo", [fp, d], f32, kind="ExternalInput")
    yT_d = nc.dram_tensor("yT", [d, bs], f32, kind="ExternalOutput")

    xT_v = xT_d.ap().rearrange("(k p) m -> k p m", p=128)      # [kt,128,bs]
    yT_v = yT_d.ap().rearrange("(n p) m -> n p m", p=128)      # [d/128,128,bs]

    dbg = {}
    if debug_outputs:
        qw0 = min(QW, s)
        for nm, shape in [("dbg_qT", [128, s]), ("dbg_kT", [128, s]),
                          ("dbg_v", [128, (s // 128) * 130]),
                          ("dbg_exp", [128, 1024]), ("dbg_poa", [65, qw0]),
                          ("dbg_pob", [65, qw0]), ("dbg_bc", [128, qw0]),
                          ("dbg_ao", [128, qw0]), ("dbg_rc", [2, qw0])]:
            dbg[nm] = nc.dram_tensor(nm, shape, f32, kind="ExternalOutput")

    with tile.TileContext(nc) as tc:
        with (
            tc.tile_pool(name="p_x", bufs=1) as p_x,
            tc.tile_pool(name="p_qkv", bufs=2) as p_qkv,
            tc.tile_pool(name="p_w", bufs=1) as p_w,
            tc.tile_pool(name="p_ao", bufs=1) as p_ao,
            tc.tile_pool(name="p_exp", bufs=3) as p_exp,
            tc.tile_pool(name="p_y", bufs=4) as p_y,
            tc.tile_pool(name="p_misc", bufs=2) as p_misc,
            tc.tile_pool(name="pp_mm", bufs=2, space="PSUM") as pp_mm,
            tc.tile_pool(name="pp_sc", bufs=2, space="PSUM") as pp_sc,
            tc.tile_pool(name="pp_o", bufs=1, space="PSUM") as pp_o,
        ):
            # ---- weights (loaded once) ----
            wq_sb = p_w.tile([128, kt, fp], f32)
            wk_sb = p_w.tile([128, kt, fp], f32)
            wv_sb = p_w.tile([128, kt, fp], f32)
            wo_sb = p_w.tile([128, d], f32)
            ones_sb = p_w.tile([128, 1], f32)
            nc.sync.dma_start(wq_sb[:], wq_d.ap().rearrange("(k p) f -> p k f", p=128))
            nc.sync.dma_start(wk_sb[:], wk_d.ap().rearrange("(k p) f -> p k f", p=128))
            nc.sync.dma_start(wv_sb[:], wv_d.ap().rearrange("(k p) f -> p k f", p=128))
            nc.sync.dma_start(wo_sb[:], wo_d.ap())
            nc.vector.memset(ones_sb[:], 1.0)

            aoT_sb = p_ao.tile([128, bs], f32)   # normalized attn outT, both batches

            for bb in range(b):
                m0 = bb * s
                # ---- load xT for this batch ----
                xTb = p_x.tile([128, kt, s], f32, tag="xTb")
                for k in range(kt):
                    nc.sync.dma_start(xTb[:, k, :], xT_v[k, :, m0:m0 + s])

                # ---- qkv projections ----
                qT_sb = p_qkv.tile([128, s], f32, tag="qT")
                kT_sb = p_qkv.tile([128, s], f32, tag="kT")
                v_sb = p_qkv.tile([128, vmb, 130], f32, tag="v")
                for ms in range(nms):
                    sl = slice(ms * msw, (ms + 1) * msw)
                    pq = pp_mm.tile([128, 512], f32, tag="mm")
                    for k in range(kt):
                        nc.tensor.matmul(pq[:, :msw], wq_sb[:, k, :], xTb[:, k, sl],
                                         start=(k == 0), stop=(k == kt - 1))
                    nc.vector.tensor_copy(qT_sb[:, sl], pq[:, :msw])
                    pk = pp_mm.tile([128, 512], f32, tag="mm")
                    for k in range(kt):
                        nc.tensor.matmul(pk[:, :msw], wk_sb[:, k, :], xTb[:, k, sl],
                                         start=(k == 0), stop=(k == kt - 1))
                    nc.vector.tensor_copy(kT_sb[:, sl], pk[:, :msw])
                # v_ext layout per m-block: [vA | 1 | vB | 1]  (65 cols per head)
                nc.vector.memset(v_sb[:, :, 64:65], 1.0)
                nc.vector.memset(v_sb[:, :, 129:130], 1.0)
                for mb in range(vmb):
                    pv = pp_mm.tile([128, 512], f32, tag="mm")
                    for k in range(kt):
                        nc.tensor.matmul(pv[:, :128], xTb[:, k, mb * 128:(mb + 1) * 128],
                                         wv_sb[:, k, :],
                                         start=(k == 0), stop=(k == kt - 1))
                    nc.vector.tensor_copy(v_sb[:, mb, 0:64], pv[:, 0:64])
                    nc.vector.tensor_copy(v_sb[:, mb, 65:129], pv[:, 64:128])

                if debug_outputs and bb == 0:
                    nc.sync.dma_start(dbg["dbg_qT"].ap(), qT_sb[:])
                    nc.sync.dma_start(dbg["dbg_kT"].ap(), kT_sb[:])
                    nc.sync.dma_start(
                        dbg["dbg_v"].ap().rearrange("p (m c) -> p m c", c=130),
                        v_sb[:])

                # ---- attention ----
                for qs in range(nqs):
                    qsl = slice(qs * qw, (qs + 1) * qw)
                    # per-head accumulators: rows 0-63 = attn@V, row 64 = colsum
                    poa = pp_o.tile([65, qw], f32, tag="poa")
                    pob = pp_o.tile([65, qw], f32, tag="pob")
                    for ki in range(ki_n):
                        ksl = slice(ki * 128, (ki + 1) * 128)
                        # scoresT: two heads row-tiled (K=64 each)
                        psc = pp_sc.tile([128, 1024], f32, tag="sc")
                        nc.tensor.matmul(psc[:, 0:qw], kT_sb[0:64, ksl],
                                         qT_sb[0:64, qsl], start=True, stop=True,
                                         tile_position=(0, 0))
                        nc.tensor.matmul(psc[:, 512:512 + qw], kT_sb[64:128, ksl],
                                         qT_sb[64:128, qsl], start=True, stop=True,
                                         tile_position=(64, 0))
                        # exp (scale=1/sqrt(dh)) straight from PSUM
                        ex = p_exp.tile([128, 1024], f32, tag="exp")
                        if qw == 512:
                            nc.scalar.activation(ex[:], psc[:], AF.Exp, scale=0.125)
                        else:
                            nc.scalar.activation(ex[:, 0:qw], psc[:, 0:qw], AF.Exp,
                                                 scale=0.125)
                            nc.scalar.activation(ex[:, 512:512 + qw],
                                                 psc[:, 512:512 + qw], AF.Exp,
                                                 scale=0.125)
                        if debug_outputs and bb == 0 and qs == 0 and ki == 0:
                            nc.sync.dma_start(dbg["dbg_exp"].ap(), ex[:])
                        st = ki == 0
                        sp = ki == ki_n - 1
                        # attnV + colsum via [v | ones] (M=65), one bank per head
                        nc.tensor.matmul(poa[:, :], v_sb[:, ki, 0:65],
                                         ex[:, 0:qw], start=st, stop=sp)
                        nc.tensor.matmul(pob[:, :], v_sb[:, ki, 65:130],
                                         ex[:, 512:512 + qw], start=st, stop=sp)
                    # normalize: aoT = aoT_un * (1/colsum).  partition_broadcast
                    # ucode writes partitions 0..channels-1 regardless of dst
                    # base, so each bcast target is its own base-0 tile; the
                    # muls use partition-base-shifted operands (HW-verified).
                    rca = p_misc.tile([1, qw], f32, tag="rca")
                    rcb = p_misc.tile([1, qw], f32, tag="rcb")
                    nc.vector.reciprocal(rca[:], poa[64:65, :])
                    nc.vector.reciprocal(rcb[:], pob[64:65, :])
                    bca = p_misc.tile([64, qw], f32, tag="bca")
                    bcb = p_misc.tile([64, qw], f32, tag="bcb")
                    nc.gpsimd.partition_broadcast(bca[:], rca[:])
                    nc.gpsimd.partition_broadcast(bcb[:], rcb[:])
                    osl = slice(m0 + qs * qw, m0 + (qs + 1) * qw)
                    if debug_outputs and bb == 0 and qs == 0:
                        pd = p_misc.tile([65, qw], f32, tag="pd")
                        nc.vector.tensor_copy(pd[:], poa[:])
                        nc.sync.dma_start(dbg["dbg_poa"].ap(), pd[:])
                        pd2 = p_misc.tile([65, qw], f32, tag="pd2")
                        nc.vector.tensor_copy(pd2[:], pob[:])
                        nc.sync.dma_start(dbg["dbg_pob"].ap(), pd2[:])
                    nc.vector.tensor_mul(aoT_sb[0:64, osl], poa[0:64, :], bca[:])
                    nc.vector.tensor_mul(aoT_sb[64:128, osl], pob[0:64, :], bcb[:])
                    if debug_outputs and bb == 0 and qs == 0:
                        nc.sync.dma_start(dbg["dbg_bc"].ap()[0:64, :], bca[:])
                        nc.sync.dma_start(dbg["dbg_bc"].ap()[64:128, :], bcb[:])
                        nc.sync.dma_start(dbg["dbg_ao"].ap(), aoT_sb[:, osl])
                        nc.sync.dma_start(dbg["dbg_rc"].ap()[0:1, :], rca[:])
                        nc.sync.dma_start(dbg["dbg_rc"].ap()[1:2, :], rcb[:])

                # ---- output projection (partial): yT = wo^T @ aoT ----
                for nb in range(d // 128):
                    for ms in range(nms):
                        sl = slice(m0 + ms * msw, m0 + (ms + 1) * msw)
                        py = pp_mm.tile([128, 512], f32, tag="mm")
                        nc.tensor.matmul(py[:, :msw], wo_sb[:, nb * 128:(nb + 1) * 128],
                                         aoT_sb[:, sl], start=True, stop=True)
                        yst = p_y.tile([128, 512], f32, tag="y")
                        nc.vector.tensor_copy(yst[:, :msw], py[:, :msw])
                        nc.sync.dma_start(yT_v[nb, :, sl], yst[:, :msw])

    nc.compile()
    return nc


_CACHE = {}


def _prep_inputs(x, w_qkv, w_out):
    """Host-side shard prep: returns per-core input maps."""
    b, s, d = x.shape
    bs = b * s
    xT = np.ascontiguousarray(x.reshape(bs, d).T.astype(np.float32))
    wq = w_qkv[:, 0 * d:1 * d].reshape(d, H, DH)
    wk = w_qkv[:, 1 * d:2 * d].reshape(d, H, DH)
    wv = w_qkv[:, 2 * d:3 * d].reshape(d, H, DH)
    in_maps = []
    for c in range(NCORES):
        hs = slice(HPC * c, HPC * (c + 1))
        in_maps.append({
            "xT": xT,
            "wq": np.ascontiguousarray(wq[:, hs, :].reshape(d, FP)).astype(np.float32),
            "wk": np.ascontiguousarray(wk[:, hs, :].reshape(d, FP)).astype(np.float32),
            "wv": np.ascontiguousarray(wv[:, hs, :].reshape(d, FP)).astype(np.float32),
            "wo": np.ascontiguousarray(w_out[FP * c:FP * (c + 1), :]).astype(np.float32),
        })
    return in_maps


class _PjrtRunner:
    """Caches the shard_map-jitted executable for a compiled Bass program so it
    can be invoked (and timed) repeatedly."""

    def __init__(self, nc, n_cores=NCORES):
        import jax
        import numpy as _np
        import concourse.mybir as mybir
        from concourse import bass2jax
        from jax.sharding import Mesh, PartitionSpec
        from jax.experimental.shard_map import shard_map

        bass2jax.install_neuronx_cc_hook()
        self.jax = jax
        self.nc = nc
        self.n_cores = n_cores
        partition_name = (nc.partition_id_tensor.name
                          if nc.partition_id_tensor else None)
        self.partition_name = partition_name
        in_names, out_names, out_avals, zero_outs = [], [], [], []
        for alloc in nc.m.functions[0].allocations:
            if not isinstance(alloc, mybir.MemoryLocationSet):
                continue
            name = alloc.memorylocations[0].name
            if alloc.kind == "ExternalInput":
                if name != partition_name:
                    in_names.append(name)
            elif alloc.kind == "ExternalOutput":
                out_names.append(name)
                shape = tuple(alloc.tensor_shape)
                dtype = mybir.dt.np(alloc.dtype)
                out_avals.append(jax.core.ShapedArray(shape, dtype))
                zero_outs.append(_np.zeros(shape, dtype))
        self.in_names, self.out_names = in_names, out_names
        self.out_avals, self.zero_outs = out_avals, zero_outs
        n_params, n_outs = len(in_names), len(out_names)
        self.n_params, self.n_outs = n_params, n_outs
        all_names = in_names + out_names
        if partition_name is not None:
            all_names = all_names + [partition_name]

        def _body(*args):
            operands = list(args)
            if partition_name is not None:
                operands.append(bass2jax.partition_id_tensor())
            outs = bass2jax._bass_exec_p.bind(
                *operands,
                out_avals=tuple(out_avals),
                in_names=tuple(all_names),
                out_names=tuple(out_names),
                lowering_input_output_aliases=(),
                sim_require_finite=True,
                sim_require_nnan=True,
                nc=nc,
            )
            return tuple(outs)

        devices = jax.devices()[:n_cores]
        assert len(devices) == n_cores
        mesh = Mesh(np.asarray(devices), ("core",))
        in_specs = (PartitionSpec("core"),) * (n_params + n_outs)
        out_specs = (PartitionSpec("core"),) * n_outs
        self.fn = jax.jit(
            shard_map(_body, mesh=mesh, in_specs=in_specs, out_specs=out_specs,
                      check_rep=False),
            donate_argnums=tuple(range(n_params, n_params + n_outs)),
            keep_unused=True,
        )
        self.mesh = mesh
        self._dev_inputs = None

    def set_inputs(self, in_maps):
        import jax
        concat_in = [
            np.concatenate([np.asarray(in_maps[c][n]) for c in range(self.n_cores)],
                           axis=0)
            for n in self.in_names
        ]
        self._dev_inputs = [jax.device_put(a) for a in concat_in]

    def _zeros(self):
        return [np.zeros((self.n_cores * z.shape[0], *z.shape[1:]), z.dtype)
                for z in self.zero_outs]

    def run(self):
        out_arrs = self.fn(*self._dev_inputs, *self._zeros())
        out_arrs = [np.asarray(o) for o in out_arrs]
        return [
            {n: out_arrs[i].reshape(self.n_cores, *self.out_avals[i].shape)[c]
             for i, n in enumerate(self.out_names)}
            for c in range(self.n_cores)
        ]

    def _timing_fn(self):
        """A second jit WITHOUT donation so buffers are reusable for bursts."""
        if not hasattr(self, "_tfn"):
            import jax
            from jax.sharding import PartitionSpec
            from jax.experimental.shard_map import shard_map
            self._tfn = jax.jit(
                shard_map(self._body, mesh=self.mesh,
                          in_specs=(PartitionSpec("core"),) * (self.n_params + self.n_outs),
                          out_specs=(PartitionSpec("core"),) * self.n_outs,
                          check_rep=False),
                keep_unused=True,
            )
            self._tzeros = [self.jax.device_put(z) for z in self._zeros()]
        return self._tfn

    def time_exec(self, iters=10, burst=16):
        """Per-exec time via async burst: (t_burst - t_1) / (burst - 1)."""
        import time
        fn = self._timing_fn()
        out = fn(*self._dev_inputs, *self._tzeros)
        self.jax.block_until_ready(out)

        def run_burst(n):
            t0 = time.perf_counter()
            outs = None
            for _ in range(n):
                outs = fn(*self._dev_inputs, *self._tzeros)
            self.jax.block_until_ready(outs)
            return time.perf_counter() - t0

        singles = [run_burst(1) for _ in range(iters)]
        bursts = [run_burst(burst) for _ in range(max(3, iters // 2))]
        singles.sort()
        bursts.sort()
        t1 = singles[len(singles) // 2]
        tb = bursts[len(bursts) // 2]
        per_exec = (tb - t1) / (burst - 1)
        return per_exec, {"single": singles, "burst": bursts, "burst_n": burst}


def _get_runner():
    if "runner" not in _CACHE:
        if "nc" not in _CACHE:
            _CACHE["nc"] = build_program()
        _CACHE["runner"] = _PjrtRunner(_CACHE["nc"])
    return _CACHE["runner"]


def run_on_hw(x, w_qkv, w_out, b_out, trace=False):
    r = _get_runner()
    in_maps = _prep_inputs(np.asarray(x), np.asarray(w_qkv), np.asarray(w_out))
    r.set_inputs(in_maps)
    results = r.run()
    acc = np.zeros((D, B * S), dtype=np.float64)
    for rr in results:
        acc += rr["yT"].astype(np.float64)
    y = acc.T.reshape(B, S, D).astype(np.float32) + np.asarray(b_out)[None, None, :]
    return y.astype(np.float32), results


def kernel(**inputs):
    y, _ = run_on_hw(inputs["x"], inputs["w_qkv"], inputs["w_out"], inputs["b_out"])
    return y


# revision 20
# speedup vs baseline: 61.2303x; 61.2303x over previous
"""Trainium2 Bass kernel for multi-head attention (B=2, S=2048, D=1024, H=16).

Sharding: tensor-parallel over heads — each of 8 cores owns 2 heads (both
batches).  Host pre-transposes x to xT [D, B*S] so every device matmul gets
operands in natural layout (no on-device transposes):

  per core (heads 2c, 2c+1):
    qT,kT [128, BS]  = w_{q,k}colsT @ x   (lhsT = w cols  [D,128], rhs = xT)
    v     [BS, 128]  = x @ w_vcols        (lhsT = xT tile [D,128], rhs = w_v)
    scoresT [ki,q]   = k qT               (lhsT = kT 64-row blocks, rhs = qT;
                                           2 heads packed via PE row-tiling)
    exp on ScalarE straight from PSUM, no max subtraction (scores ~ N(0,1))
    aoT_un [128f, q] = v^T exp            (2 heads packed via PE col-tiling)
    colsum [1, q]    = ones^T exp         (concurrent M=1 matmuls)
    aoT = aoT_un * (1/colsum broadcast)   (gpsimd partition_broadcast + DVE)
    yT_partial [D, BS] = w_out_rowsT @ aoT

  host: y = (sum_c yT_partial_c).T.reshape(B,S,D) + b_out
"""

import os
import sys

for _p in ("/opt/trn_rl_repo",):
    if _p not in sys.path and os.path.isdir(_p):
        sys.path.insert(0, _p)

import numpy as np

# Problem shapes (hardcoded per contest rules).
B, S, D, H = 2, 2048, 1024, 16
DH = D // H            # 64
NCORES = 8
HPC = H // NCORES      # 2 heads per core
FP = HPC * DH          # 128 features per core
BS = B * S             # 4096 tokens
KT = D // 128          # 8 contraction k-tiles
QW = 512               # attention q-slice width (one PSUM bank of fp32)


def build_program(b=B, s=S, d=D, h=H, ncores=NCORES, debug_outputs=False):
    """Build + compile the per-core Bass program. Same program for all cores
    (SPMD); per-core differences come in through the weight slices."""
    import concourse.bass as bass
    import concourse.mybir as mybir
    import concourse.tile as tile
    from concourse import bacc

    dh = d // h
    hpc = h // ncores
    fp = hpc * dh                      # per-core features (q/k/v width)
    bs = b * s
    kt = d // 128                      # k tiles over model dim
    qw = min(QW, s)                    # q-slice width
    nqs = s // qw                      # q slices per batch
    ki_n = s // 128                    # key blocks per batch
    vmb = s // 128                     # v m-blocks per batch
    nms = s // 512 if s >= 512 else 1  # qkv m-slices per batch
    msw = min(512, s)                  # m-slice width
    assert fp == 128 and dh == 64

    f32 = mybir.dt.float32
    AF = mybir.ActivationFunctionType

    nc = bacc.Bacc("TRN2", target_bir_lowering=False, debug=False)

    xT_d = nc.dram_tensor("xT", [d, bs], f32, kind="ExternalInput")
    wq_d = nc.dram_tensor("wq", [d, fp], f32, kind="ExternalInput")
    wk_d = nc.dram_tensor("wk", [d, fp], f32, kind="ExternalInput")
    wv_d = nc.dram_tensor("wv", [d, fp], f32, kind="ExternalInput")
    wo_d = nc.dram_tensor("wo", [fp, d], f32, kind="ExternalInput")
    yT_d = nc.dram_tensor("yT", [d, bs], f32, kind="ExternalOutput")

    xT_v = xT_d.ap().rearrange("(k p) m -> k p m", p=128)      # [kt,128,bs]
    yT_v = yT_d.ap().rearrange("(n p) m -> n p m", p=128)      # [d/128,128,bs]

    dbg = {}
    if debug_outputs:
        qw0 = min(QW, s)
        for nm, shape in [("dbg_qT", [128, s]), ("dbg_kT", [128, s]),
                          ("dbg_v", [128, (s // 128) * 130]),
                          ("dbg_exp", [128, 1024]), ("dbg_poa", [65, qw0]),
                          ("dbg_pob", [65, qw0]), ("dbg_bc", [128, qw0]),
                          ("dbg_ao", [128, qw0]), ("dbg_rc", [2, qw0])]:
            dbg[nm] = nc.dram_tensor(nm, shape, f32, kind="ExternalOutput")

    with tile.TileContext(nc) as tc:
        with (
            tc.tile_pool(name="p_x", bufs=1) as p_x,
            tc.tile_pool(name="p_qkv", bufs=2) as p_qkv,
            tc.tile_pool(name="p_w", bufs=1) as p_w,
            tc.tile_pool(name="p_ao", bufs=1) as p_ao,
            tc.tile_pool(name="p_exp", bufs=3) as p_exp,
            tc.tile_pool(name="p_y", bufs=4) as p_y,
            tc.tile_pool(name="p_misc", bufs=2) as p_misc,
            tc.tile_pool(name="pp_mm", bufs=2, space="PSUM") as pp_mm,
            tc.tile_pool(name="pp_sc", bufs=2, space="PSUM") as pp_sc,
            tc.tile_pool(name="pp_o", bufs=1, space="PSUM") as pp_o,
        ):
            # ---- weights (loaded once) ----
            wq_sb = p_w.tile([128, kt, fp], f32)
            wk_sb = p_w.tile([128, kt, fp], f32)
            wv_sb = p_w.tile([128, kt, fp], f32)
            wo_sb = p_w.tile([128, d], f32)
            ones_sb = p_w.tile([128, 1], f32)
            nc.sync.dma_start(wq_sb[:], wq_d.ap().rearrange("(k p) f -> p k f", p=128))
            nc.sync.dma_start(wk_sb[:], wk_d.ap().rearrange("(k p) f -> p k f", p=128))
            nc.sync.dma_start(wv_sb[:], wv_d.ap().rearrange("(k p) f -> p k f", p=128))
            nc.sync.dma_start(wo_sb[:], wo_d.ap())
            nc.vector.memset(ones_sb[:], 1.0)

            aoT_sb = p_ao.tile([128, bs], f32)   # normalized attn outT, both batches

            for bb in range(b):
                m0 = bb * s
                # ---- load xT for this batch ----
                xTb = p_x.tile([128, kt, s], f32, tag="xTb")
                for k in range(kt):
                    nc.sync.dma_start(xTb[:, k, :], xT_v[k, :, m0:m0 + s])

                # ---- qkv projections ----
                qT_sb = p_qkv.tile([128, s], f32, tag="qT")
                kT_sb = p_qkv.tile([128, s], f32, tag="kT")
                v_sb = p_qkv.tile([128, vmb, 130], f32, tag="v")
                for ms in range(nms):
                    sl = slice(ms * msw, (ms + 1) * msw)
                    pq = pp_mm.tile([128, 512], f32, tag="mm")
                    for k in range(kt):
                        nc.tensor.matmul(pq[:, :msw], wq_sb[:, k, :], xTb[:, k, sl],
                                         start=(k == 0), stop=(k == kt - 1))
                    nc.vector.tensor_copy(qT_sb[:, sl], pq[:, :msw])
                    pk = pp_mm.tile([128, 512], f32, tag="mm")
                    for k in range(kt):
                        nc.tensor.matmul(pk[:, :msw], wk_sb[:, k, :], xTb[:, k, sl],
                                         start=(k == 0), stop=(k == kt - 1))
                    nc.vector.tensor_copy(kT_sb[:, sl], pk[:, :msw])
                # v_ext layout per m-block: [vA | 1 | vB | 1]  (65 cols per head)
                nc.vector.memset(v_sb[:, :, 64:65], 1.0)
                nc.vector.memset(v_sb[:, :, 129:130], 1.0)
                for mb in range(vmb):
                    pv = pp_mm.tile([128, 512], f32, tag="mm")
                    for k in range(kt):
                        nc.tensor.matmul(pv[:, :128], xTb[:, k, mb * 128:(mb + 1) * 128],
                                         wv_sb[:, k, :],
                                         start=(k == 0), stop=(k == kt - 1))
                    nc.vector.tensor_copy(v_sb[:, mb, 0:64], pv[:, 0:64])
                    nc.vector.tensor_copy(v_sb[:, mb, 65:129], pv[:, 64:128])

                if debug_outputs and bb == 0:
                    nc.sync.dma_start(dbg["dbg_qT"].ap(), qT_sb[:])
                    nc.sync.dma_start(dbg["dbg_kT"].ap(), kT_sb[:])
                    nc.sync.dma_start(
                        dbg["dbg_v"].ap().rearrange("p (m c) -> p m c", c=130),
                        v_sb[:])

                # ---- attention ----
                for qs in range(nqs):
                    qsl = slice(qs * qw, (qs + 1) * qw)
                    # per-head accumulators: rows 0-63 = attn@V, row 64 = colsum
                    poa = pp_o.tile([65, qw], f32, tag="poa")
                    pob = pp_o.tile([65, qw], f32, tag="pob")
                    for ki in range(ki_n):
                        ksl = slice(ki * 128, (ki + 1) * 128)
                        # scoresT: two heads row-tiled (K=64 each)
                        psc = pp_sc.tile([128, 1024], f32, tag="sc")
                        nc.tensor.matmul(psc[:, 0:qw], kT_sb[0:64, ksl],
                                         qT_sb[0:64, qsl], start=True, stop=True,
                                         tile_position=(0, 0))
                        nc.tensor.matmul(psc[:, 512:512 + qw], kT_sb[64:128, ksl],
                                         qT_sb[64:128, qsl], start=True, stop=True,
                                         tile_position=(64, 0))
                        # exp (scale=1/sqrt(dh)) straight from PSUM
                        ex = p_exp.tile([128, 1024], f32, tag="exp")
                        if qw == 512:
                            nc.scalar.activation(ex[:], psc[:], AF.Exp, scale=0.125)
                        else:
                            nc.scalar.activation(ex[:, 0:qw], psc[:, 0:qw], AF.Exp,
                                                 scale=0.125)
                            nc.scalar.activation(ex[:, 512:512 + qw],
                                                 psc[:, 512:512 + qw], AF.Exp,
                                                 scale=0.125)
                        if debug_outputs and bb == 0 and qs == 0 and ki == 0:
                            nc.sync.dma_start(dbg["dbg_exp"].ap(), ex[:])
                        st = ki == 0
                        sp = ki == ki_n - 1
                        # attnV + colsum via [v | ones] (M=65), one bank per head
                        nc.tensor.matmul(poa[:, :], v_sb[:, ki, 0:65],
                                         ex[:, 0:qw], start=st, stop=sp)
                        nc.tensor.matmul(pob[:, :], v_sb[:, ki, 65:130],
                                         ex[:, 512:512 + qw], start=st, stop=sp)
                    # normalize: aoT = aoT_un * (1/colsum).  partition_broadcast
                    # ucode writes partitions 0..channels-1 regardless of dst
                    # base, so each bcast target is its own base-0 tile; the
                    # muls use partition-base-shifted operands (HW-verified).
                    rca = p_misc.tile([1, qw], f32, tag="rca")
                    rcb = p_misc.tile([1, qw], f32, tag="rcb")
                    nc.vector.reciprocal(rca[:], poa[64:65, :])
                    nc.vector.reciprocal(rcb[:], pob[64:65, :])
                    bca = p_misc.tile([64, qw], f32, tag="bca")
                    bcb = p_misc.tile([64, qw], f32, tag="bcb")
                    nc.gpsimd.partition_broadcast(bca[:], rca[:])
                    nc.gpsimd.partition_broadcast(bcb[:], rcb[:])
                    osl = slice(m0 + qs * qw, m0 + (qs + 1) * qw)
                    if debug_outputs and bb == 0 and qs == 0:
                        pd = p_misc.tile([65, qw], f32, tag="pd")
                        nc.vector.tensor_copy(pd[:], poa[:])
                        nc.sync.dma_start(dbg["dbg_poa"].ap(), pd[:])
                        pd2 = p_misc.tile([65, qw], f32, tag="pd2")
                        nc.vector.tensor_copy(pd2[:], pob[:])
                        nc.sync.dma_start(dbg["dbg_pob"].ap(), pd2[:])
                    nc.vector.tensor_mul(aoT_sb[0:64, osl], poa[0:64, :], bca[:])
                    nc.vector.tensor_mul(aoT_sb[64:128, osl], pob[0:64, :], bcb[:])
                    if debug_outputs and bb == 0 and qs == 0:
                        nc.sync.dma_start(dbg["dbg_bc"].ap()[0:64, :], bca[:])
                        nc.sync.dma_start(dbg["dbg_bc"].ap()[64:128, :], bcb[:])
                        nc.sync.dma_start(dbg["dbg_ao"].ap(), aoT_sb[:, osl])
                        nc.sync.dma_start(dbg["dbg_rc"].ap()[0:1, :], rca[:])
                        nc.sync.dma_start(dbg["dbg_rc"].ap()[1:2, :], rcb[:])

                # ---- output projection (partial): yT = wo^T @ aoT ----
                for nb in range(d // 128):
                    for ms in range(nms):
                        sl = slice(m0 + ms * msw, m0 + (ms + 1) * msw)
                        py = pp_mm.tile([128, 512], f32, tag="mm")
                        nc.tensor.matmul(py[:, :msw], wo_sb[:, nb * 128:(nb + 1) * 128],
                                         aoT_sb[:, sl], start=True, stop=True)
                        yst = p_y.tile([128, 512], f32, tag="y")
                        nc.vector.tensor_copy(yst[:, :msw], py[:, :msw])
                        nc.sync.dma_start(yT_v[nb, :, sl], yst[:, :msw])

    nc.compile()
    return nc


_CACHE = {}


def _prep_inputs(x, w_qkv, w_out):
    """Host-side shard prep: returns per-core input maps."""
    b, s, d = x.shape
    bs = b * s
    xT = np.ascontiguousarray(x.reshape(bs, d).T.astype(np.float32))
    wq = w_qkv[:, 0 * d:1 * d].reshape(d, H, DH)
    wk = w_qkv[:, 1 * d:2 * d].reshape(d, H, DH)
    wv = w_qkv[:, 2 * d:3 * d].reshape(d, H, DH)
    in_maps = []
    for c in range(NCORES):
        hs = slice(HPC * c, HPC * (c + 1))
        in_maps.append({
            "xT": xT,
            "wq": np.ascontiguousarray(wq[:, hs, :].reshape(d, FP)).astype(np.float32),
            "wk": np.ascontiguousarray(wk[:, hs, :].reshape(d, FP)).astype(np.float32),
            "wv": np.ascontiguousarray(wv[:, hs, :].reshape(d, FP)).astype(np.float32),
            "wo": np.ascontiguousarray(w_out[FP * c:FP * (c + 1), :]).astype(np.float32),
        })
    return in_maps


class _PjrtRunner:
    """Caches the shard_map-jitted executable for a compiled Bass program so it
    can be invoked (and timed) repeatedly."""

    def __init__(self, nc, n_cores=NCORES):
        import jax
        import numpy as _np
        import concourse.mybir as mybir
        from concourse import bass2jax
        from jax.sharding import Mesh, PartitionSpec
        from jax.experimental.shard_map import shard_map

        bass2jax.install_neuronx_cc_hook()
        self.jax = jax
        self.nc = nc
        self.n_cores = n_cores
        partition_name = (nc.partition_id_tensor.name
                          if nc.partition_id_tensor else None)
        self.partition_name = partition_name
        in_names, out_names, out_avals, zero_outs = [], [], [], []
        for alloc in nc.m.functions[0].allocations:
            if not isinstance(alloc, mybir.MemoryLocationSet):
                continue
            name = alloc.memorylocations[0].name
            if alloc.kind == "ExternalInput":
                if name != partition_name:
                    in_names.append(name)
            elif alloc.kind == "ExternalOutput":
                out_names.append(name)
                shape = tuple(alloc.tensor_shape)
                dtype = mybir.dt.np(alloc.dtype)
                out_avals.append(jax.core.ShapedArray(shape, dtype))
                zero_outs.append(_np.zeros(shape, dtype))
        self.in_names, self.out_names = in_names, out_names
        self.out_avals, self.zero_outs = out_avals, zero_outs
        n_params, n_outs = len(in_names), len(out_names)
        self.n_params, self.n_outs = n_params, n_outs
        all_names = in_names + out_names
        if partition_name is not None:
            all_names = all_names + [partition_name]

        def _body(*args):
            operands = list(args)
            if partition_name is not None:
                operands.append(bass2jax.partition_id_tensor())
            outs = bass2jax._bass_exec_p.bind(
                *operands,
                out_avals=tuple(out_avals),
                in_names=tuple(all_names),
                out_names=tuple(out_names),
                lowering_input_output_aliases=(),
                sim_require_finite=True,
                sim_require_nnan=True,
                nc=nc,
            )
            return tuple(outs)

        self._body = _body
        devices = jax.devices()[:n_cores]
        assert len(devices) == n_cores
        mesh = Mesh(np.asarray(devices), ("core",))
        in_specs = (PartitionSpec("core"),) * (n_params + n_outs)
        out_specs = (PartitionSpec("core"),) * n_outs
        self.fn = jax.jit(
            shard_map(_body, mesh=mesh, in_specs=in_specs, out_specs=out_specs,
                      check_rep=False),
            donate_argnums=tuple(range(n_params, n_params + n_outs)),
            keep_unused=True,
        )
        self.mesh = mesh
        self._dev_inputs = None

    def set_inputs(self, in_maps):
        import jax
        concat_in = [
            np.concatenate([np.asarray(in_maps[c][n]) for c in range(self.n_cores)],
                           axis=0)
            for n in self.in_names
        ]
        self._dev_inputs = [jax.device_put(a) for a in concat_in]

    def _zeros(self):
        return [np.zeros((self.n_cores * z.shape[0], *z.shape[1:]), z.dtype)
                for z in self.zero_outs]

    def run(self):
        out_arrs = self.fn(*self._dev_inputs, *self._zeros())
        out_arrs = [np.asarray(o) for o in out_arrs]
        return [
            {n: out_arrs[i].reshape(self.n_cores, *self.out_avals[i].shape)[c]
             for i, n in enumerate(self.out_names)}
            for c in range(self.n_cores)
        ]

    def _timing_fn(self):
        """A second jit WITHOUT donation so buffers are reusable for bursts."""
        if not hasattr(self, "_tfn"):
            import jax
            from jax.sharding import PartitionSpec
            from jax.experimental.shard_map import shard_map
            self._tfn = jax.jit(
                shard_map(self._body, mesh=self.mesh,
                          in_specs=(PartitionSpec("core"),) * (self.n_params + self.n_outs),
                          out_specs=(PartitionSpec("core"),) * self.n_outs,
                          check_rep=False),
                keep_unused=True,
            )
            self._tzeros = [self.jax.device_put(z) for z in self._zeros()]
        return self._tfn

    def time_exec(self, iters=10, burst=16):
        """Per-exec time via async burst: (t_burst - t_1) / (burst - 1)."""
        import time
        fn = self._timing_fn()
        out = fn(*self._dev_inputs, *self._tzeros)
        self.jax.block_until_ready(out)

        def run_burst(n):
            t0 = time.perf_counter()
            outs = None
            for _ in range(n):
                outs = fn(*self._dev_inputs, *self._tzeros)
            self.jax.block_until_ready(outs)
            return time.perf_counter() - t0

        singles = [run_burst(1) for _ in range(iters)]
        bursts = [run_burst(burst) for _ in range(max(3, iters // 2))]
        singles.sort()
        bursts.sort()
        t1 = singles[len(singles) // 2]
        tb = bursts[len(bursts) // 2]
        per_exec = (tb - t1) / (burst - 1)
        return per_exec, {"single": singles, "burst": bursts, "burst_n": burst}


def _get_runner():
    if "runner" not in _CACHE:
        if "nc" not in _CACHE:
            _CACHE["nc"] = build_program()
        _CACHE["runner"] = _PjrtRunner(_CACHE["nc"])
    return _CACHE["runner"]


def run_on_hw(x, w_qkv, w_out, b_out, trace=False):
    r = _get_runner()
    in_maps = _prep_inputs(np.asarray(x), np.asarray(w_qkv), np.asarray(w_out))
    r.set_inputs(in_maps)
    results = r.run()
    acc = np.zeros((D, B * S), dtype=np.float64)
    for rr in results:
        acc += rr["yT"].astype(np.float64)
    y = acc.T.reshape(B, S, D).astype(np.float32) + np.asarray(b_out)[None, None, :]
    return y.astype(np.float32), results


def kernel(**inputs):
    y, _ = run_on_hw(inputs["x"], inputs["w_qkv"], inputs["w_out"], inputs["b_out"])
    return y
